# revision 14
# baseline (speedup 1.0000x reference)
"""Connectome kernel (segment-mean -> Pearson Gram) for 8 TRN2 NeuronCores.

Strategy (pure data parallel, 2 samples per core):
  - Host marshalling: fold mask into parcellation; DROP background /
    masked-out pixels (~50% of V) entirely; sort surviving pixels by ROI
    and pack them into 128-pixel chunks (block B = ROIs 128..199 FIRST,
    then block A = ROIs 0..127; each block padded to a chunk boundary
    with label -1 slots). x is gathered into this packed order, cast
    fp16, laid out [p, chunk, sample, t] per core so each SBUF partition
    reads one contiguous HBM run per chunk-tile. Wire traffic per core:
    ~18.3MB (vs 73.7MB for fp32 all-pixels).
  - Device: stream chunk-tiles on the two HWDGE rings, tiles assigned
    to rings greedily by cumulative bytes so both rings finish together;
    onehots per tile are built in batched DVE tensor_tensor ops
    (is_equal of broadcast iota vs broadcast labels); per chunk one PE
    matmul acc[r, row] += onehot.T @ x_chunk (fp16 operands, fp32 PSUM).
  - Centering cancels analytically: C C^T = S S^T - (1/T) m m^T with
    m = per-ROI row sums, so the device Grams the RAW sums S and ships
    the tiny row sums; the host applies the rank-1 correction and the
    1/norm scaling (norms from the corrected diagonal). The /counts
    ROI-mean scaling and the +eps normalizer cancel in the Pearson Gram.
  - Per block: one DVE cast of the PSUM sums to fp16 + one row-sum
    reduce, then PE transposes into [t, roi] tiles. Block B's work
    overlaps block A's stream; the tail is block A's cast + transposes,
    8 Gram matmuls (both samples packed per PSUM bank), two wide fp16
    conn DMAs.
  - Host: concat cores, rank-1 correct, normalize, upper triangle
    -> (16, 19900) fp32.
"""
import sys

sys.path.insert(0, "/opt/trn_rl_repo")

import numpy as np

import concourse.bass as bass
import concourse.tile as tile
from concourse import bacc, mybir
from concourse.bass_utils import run_bass_kernel_spmd

F32 = mybir.dt.float32
F16 = mybir.dt.float16

N, T, H, W = 16, 200, 144, 320
V = H * W                      # 46080
R = 200                        # ROIs
RA = 128                       # ROI block A width (ROIs 0..127)
RB = R - RA                    # ROI block B width (72; ROIs 128..199)
NCORES = 8
SPB = N // NCORES              # samples per core = 2
ROWS = SPB * T                 # 400
EPS = 1e-8


def _tile_sizes(nch):
    """DMA tile schedule: small first tiles to fill the pipe fast, 8s in
    steady state, small tapered tiles at the end so the PE drain after
    the last transfer is short."""
    sizes, left = [], nch
    while left and len(sizes) < 4:
        ct = min(4, left)
        sizes.append(ct)
        left -= ct
    while left >= 16:
        sizes.append(8)
        left -= 8
    if left > 8:
        sizes.append(left - 8)
        left = 8
    while left:
        ct = min(4, left)
        sizes.append(ct)
        left -= ct
    return sizes


_cached = {}


def _bc3(ap2, ins_pos, n):
    """Insert a broadcast (stride 0, count n) dim into a 2D AP."""
    layout = [list(d) for d in ap2.ap]
    layout.insert(ins_pos, [0, n])
    return bass.AP(ap2.tensor, ap2.offset, layout)


def _split_st(ap2):
    """View a [P, SPB*T] AP as [P, SPB, T] (split the free dim)."""
    layout = [list(d) for d in ap2.ap]
    assert layout[-1][0] == 1 and layout[-1][1] == SPB * T
    layout = layout[:-1] + [[T, SPB], [1, T]]
    return bass.AP(ap2.tensor, ap2.offset, layout)


def _build_program(nA, nB):
    nch = nA + nB
    nc = bacc.Bacc("TRN2", target_bir_lowering=False, debug=False)

    # consts packed into one DRAM tensor: cols [0:nch] labs, [nch:+128]
    # iota, [+128:+256] i128, [+256:+328] i72 (partitions 72:128 zero).
    CC = nch + 328
    x_d = nc.declare_dram_parameter("x", [128, nch, ROWS], F16, isOutput=False)
    cst_d = nc.declare_dram_parameter("consts", [128, CC], F16, isOutput=False)
    # conn2 cols: [0:200] G_s0 rois 0:128, [200:400] G_s1 rois 0:128,
    # [400:600] G_s0 rois 128:200 (parts 0:72), [600:800] G_s1 rois 128:200.
    out_d = nc.declare_dram_parameter("conn2", [128, 4 * R], F16, isOutput=True)
    msa_d = nc.declare_dram_parameter("msa", [RA, SPB], F32, isOutput=True)
    msb_d = nc.declare_dram_parameter("msb", [RB, SPB], F32, isOutput=True)

    tsizes = _tile_sizes(nch)
    # greedy byte-balanced ring assignment (sync starts with the consts)
    ring_bytes = {0: CC * 2.0, 1: 0.0}      # 0 = sync, 1 = scalar
    ring_of = []
    for ct in tsizes:
        r = 0 if ring_bytes[0] <= ring_bytes[1] else 1
        ring_of.append(r)
        ring_bytes[r] += ct * ROWS * 2.0

    with tile.TileContext(nc) as tc:
        with tc.tile_pool(name="consts", bufs=1) as consts, \
             tc.tile_pool(name="loads", bufs=3) as loads, \
             tc.tile_pool(name="ohp", bufs=1) as ohp, \
             tc.tile_pool(name="epi", bufs=1) as epi, \
             tc.tile_pool(name="psum", bufs=1, space="PSUM") as psum:

            cst_s = consts.tile([128, CC], F16)
            # two consts DMAs: labs+iota (gates the first onehot build)
            # land first; the identities aren't needed until mid-stream.
            nc.sync.dma_start(cst_s[:, 0:nch + 128], cst_d[:, 0:nch + 128])
            nc.sync.dma_start(cst_s[:, nch + 128:CC], cst_d[:, nch + 128:CC])
            labs_s = cst_s[:, 0:nch]
            iota_s = cst_s[:, nch:nch + 128]
            i128_s = cst_s[:, nch + 128:nch + 256]
            i72_s = cst_s[0:72, nch + 256:nch + 328]

            acc_a = psum.tile([RA, ROWS], F32, tag="acc_a", bufs=1)
            acc_b = psum.tile([RB, ROWS], F32, tag="acc_b", bufs=1)

            # PSUM tr tiles: [t-block, roi] transposed raw-sum rows.
            tr = {}
            for s in range(SPB):
                tr[("A", s)] = psum.tile([128, R], F16, tag="trA", bufs=2,
                                         name=f"trA_{s}")
                tr[("B", s)] = psum.tile([72, R], F16, tag="trB", bufs=2,
                                         name=f"trB_{s}")

            def finish_block(blk, acc, P, ms_d, ms_eng):
                """Raw-sum epilogue for one ROI block: cast the PSUM sums
                to fp16 (Gram/transpose operand) and ship per-sample row
                sums (host applies the rank-1 centering correction)."""
                S16 = epi.tile([P, ROWS], F16, tag=f"S16_{blk}")
                ms = epi.tile([P, SPB], F32, tag=f"ms_{blk}")
                nc.vector.tensor_copy(S16[:], acc[:])
                nc.vector.tensor_reduce(ms[:], _split_st(acc[:]),
                                        axis=mybir.AxisListType.X,
                                        op=mybir.AluOpType.add)
                ms_eng.dma_start(ms_d[:], ms[:])
                return S16

            S16_b = None
            with nc.named_scope("main"):
                ch0 = 0
                for ti, ct in enumerate(tsizes):
                    ld = loads.tile([128, ct, ROWS], F16, tag=f"ld{ct}",
                                    bufs=(16 if ct == 8 else 4),
                                    name=f"ld_{ti}")
                    eng = nc.sync if ring_of[ti] == 0 else nc.scalar
                    eng.dma_start(ld[:], x_d[:, ch0:ch0 + ct, :])

                    # batched per-tile onehot builds (DVE), one per block
                    # segment present in this tile
                    nb_i = max(0, min(nB, ch0 + ct) - ch0)       # B chunks
                    na_i = ct - nb_i                             # A chunks
                    ohB_t = ohA_t = None
                    if nb_i:
                        ohB_t = ohp.tile([128, nb_i, RB], F16,
                                         tag=f"ohB{nb_i}", bufs=4,
                                         name=f"ohB_{ti}")
                        nc.vector.tensor_tensor(
                            ohB_t[:], _bc3(iota_s[:, 0:RB], 1, nb_i),
                            _bc3(labs_s[:, ch0:ch0 + nb_i], 2, RB),
                            op=mybir.AluOpType.is_equal)
                    if na_i:
                        a0 = ch0 + nb_i
                        ohA_t = ohp.tile([128, na_i, RA], F16,
                                         tag=f"ohA{na_i}", bufs=4,
                                         name=f"ohA_{ti}")
                        nc.vector.tensor_tensor(
                            ohA_t[:], _bc3(iota_s[:, 0:RA], 1, na_i),
                            _bc3(labs_s[:, a0:a0 + na_i], 2, RA),
                            op=mybir.AluOpType.is_equal)

                    for j in range(ct):
                        cc = ch0 + j
                        if cc < nB:
                            acc, oh = acc_b, ohB_t[:, j, :]
                            start, stop = (cc == 0), (cc == nB - 1)
                        else:
                            acc, oh = acc_a, ohA_t[:, j - nb_i, :]
                            start, stop = (cc == nB), (cc == nch - 1)
                        nc.tensor.matmul(acc[:], oh, ld[:, j, :],
                                         start=start, stop=stop)
                    ch0 += ct

                    if ch0 - ct < nB <= ch0:
                        # block B complete: cast + row sums on DVE while
                        # block A still streams.
                        b_done_ti = ti
                        S16_b = finish_block("b", acc_b, RB, msb_d, nc.sync)
                    if S16_b is not None and ti == b_done_ti + 3:
                        # B-sourced transposes, emitted a few tiles later
                        # so the cast has finished and PE's FIFO never
                        # blocks on it.
                        for s in range(SPB):
                            nc.tensor.transpose(
                                tr[("A", s)][:, 128:200],
                                S16_b[:, s * T:s * T + 128], i72_s)
                            nc.tensor.transpose(
                                tr[("B", s)][:, 128:200],
                                S16_b[:, s * T + 128:s * T + 200], i72_s)

            with nc.named_scope("epilogue"):
                # block-A finish: casts split per sample so s0's transposes
                # start half a cast earlier; the row-sum reduce runs before
                # the Grams so acc_a's bank can be reused for Gram s0.
                S16_a = epi.tile([RA, ROWS], F16, tag="S16_a")
                ms_a = epi.tile([RA, SPB], F32, tag="ms_a")
                tr_sb = {}
                for s in range(SPB):
                    nc.vector.tensor_copy(S16_a[:, s * T:(s + 1) * T],
                                          acc_a[:, s * T:(s + 1) * T])
                    nc.tensor.transpose(tr[("A", s)][:, 0:128],
                                        S16_a[:, s * T:s * T + 128], i128_s)
                    nc.tensor.transpose(tr[("B", s)][:, 0:128],
                                        S16_a[:, s * T + 128:s * T + 200],
                                        i128_s)
                # row-sum reduce directly after the casts in the DVE queue:
                # it is acc_a's last reader, and sample 1's Grams reuse
                # that bank.
                nc.vector.tensor_reduce(ms_a[:], _split_st(acc_a[:]),
                                        axis=mybir.AxisListType.X,
                                        op=mybir.AluOpType.add)
                nc.sync.dma_start(msa_d[:], ms_a[:])
                for s in range(SPB):
                    trA_sb = epi.tile([128, R], F16, name=f"trAs_{s}",
                                      tag="trAs", bufs=2)
                    trB_sb = epi.tile([72, R], F16, name=f"trBs_{s}",
                                      tag="trBs", bufs=2)
                    nc.vector.tensor_copy(trA_sb[:], tr[("A", s)][:])
                    nc.vector.tensor_copy(trB_sb[:], tr[("B", s)][:])
                    tr_sb[s] = (trA_sb, trB_sb)

                # Gram: conn = S_t.T @ S_t (contraction over t, fp16);
                # four independent PSUM banks (sample 0 reuses the freed
                # acc_a/acc_b banks) so no Gram matmul ever waits on a
                # cast reading another sample's bank. cB DMAs ship all
                # 128 partitions (rows 72:128 are junk the host ignores)
                # - full-height transfers issue ~2x faster than 72-row.
                cA0 = psum.tile([128, R], F32, tag="cA1", name="cA0")
                cB0 = psum.tile([72, R], F32, tag="cB1", name="cB0")
                cgram = {("A", 0): cA0, ("B", 0): cB0,
                         ("A", 1): acc_a, ("B", 1): acc_b}
                connsb = epi.tile([128, 4 * R], F16, tag="connsb")
                for s in range(SPB):
                    trA_sb, trB_sb = tr_sb[s]
                    c = cgram[("A", s)]
                    nc.tensor.matmul(c[:, 0:R], trA_sb[:, 0:128], trA_sb[:],
                                     start=True, stop=False)
                    nc.tensor.matmul(c[:, 0:R], trB_sb[:, 0:128], trB_sb[:],
                                     start=False, stop=True)
                    nc.vector.tensor_copy(connsb[:, s * R:(s + 1) * R],
                                          c[:, 0:R])
                    nc.sync.dma_start(out_d[:, s * R:(s + 1) * R],
                                      connsb[:, s * R:(s + 1) * R])
                for s in range(SPB):
                    trA_sb, trB_sb = tr_sb[s]
                    c = cgram[("B", s)]
                    nc.tensor.matmul(c[:, 0:R], trA_sb[:, 128:200], trA_sb[:],
                                     start=True, stop=False)
                    nc.tensor.matmul(c[:, 0:R], trB_sb[:, 128:200], trB_sb[:],
                                     start=False, stop=True)
                    nc.vector.tensor_copy(connsb[0:72, (2 + s) * R:(3 + s) * R],
                                          c[:, 0:R])
                    nc.scalar.dma_start(out_d[:, (2 + s) * R:(3 + s) * R],
                                        connsb[:, (2 + s) * R:(3 + s) * R])

    nc.compile()
    return nc


def _get_program(nA, nB):
    key = (nA, nB)
    if key not in _cached:
        _cached[key] = _build_program(nA, nB)
    return _cached[key]


def marshal_inputs(x, parc, mask):
    """Host-side prep: packed ROI-sorted fp16 x + tiny derived constants."""
    parc_eff = np.where(np.asarray(mask), np.asarray(parc), 0).reshape(V)
    lab = parc_eff.astype(np.int64) - 1          # -1 = dropped
    counts = np.bincount(parc_eff.astype(np.int64), minlength=R + 1)[1:]

    order = np.argsort(lab, kind="stable")
    nbg = int((lab < 0).sum())
    sorted_idx = order[nbg:]                     # kept pixels, ROI-ascending
    cA = int(counts[0:RA].sum())
    cB = int(counts[RA:R].sum())
    nA = (cA + 127) // 128
    nB = (cB + 127) // 128

    # Block B (ROIs 128..199) first, then block A.
    gB = np.concatenate([sorted_idx[cA:],
                         np.zeros(nB * 128 - cB, dtype=np.int64)])
    gA = np.concatenate([sorted_idx[:cA],
                         np.zeros(nA * 128 - cA, dtype=np.int64)])
    g = np.concatenate([gB, gA])                 # (nch*128,) gather indices
    labB = np.concatenate([lab[sorted_idx[cA:]] - RA,
                           np.full(nB * 128 - cB, -1, dtype=np.int64)])
    labA = np.concatenate([lab[sorted_idx[:cA]],
                           np.full(nA * 128 - cA, -1, dtype=np.int64)])
    nch = nA + nB
    labs = np.concatenate([labB, labA]).astype(np.float16)
    labs = labs.reshape(nch, 128).T.copy()       # (128, nch)

    iota = np.broadcast_to(np.arange(128, dtype=np.float16),
                           (128, 128)).copy()    # iota[p, c] = c
    i128 = np.eye(128, dtype=np.float16)
    i72 = np.zeros((128, 72), dtype=np.float16)
    i72[:72] = np.eye(72, dtype=np.float16)
    consts = np.concatenate([labs, iota, i128, i72], axis=1)  # (128, nch+328)

    # (N,1,T,H,W) fp32 -> packed (core, 128, nch, SPB*T) fp16
    x16 = np.asarray(x, dtype=np.float32).reshape(N, T, V).astype(np.float16)
    xg = x16[:, :, g]                            # (N, T, nch*128)
    xg = xg.reshape(NCORES, SPB, T, nch, 128)
    xs = np.ascontiguousarray(xg.transpose(0, 4, 3, 1, 2))  # (8,128,nch,2,T)
    xs = xs.reshape(NCORES, 128, nch, ROWS)

    in_maps = []
    for c in range(NCORES):
        in_maps.append({"x": xs[c], "consts": consts})
    return in_maps, nA, nB, counts


def kernel(x, parc, mask):
    in_maps, nA, nB, counts = marshal_inputs(x, parc, mask)
    nc = _get_program(nA, nB)
    res = run_bass_kernel_spmd(nc, in_maps, core_ids=list(range(NCORES)))
    # device emits the raw-sum Gram (fp16) + per-sample row sums; the
    # centering is a host-side rank-1 correction (C C^T = S S^T - m m^T/T
    # with m = row sums), and normalization a rank-1 scaling.
    G = np.empty((NCORES, SPB, R, R), np.float64)
    for c, r in enumerate(res.results):
        c2 = r["conn2"].astype(np.float64)       # (128, 800)
        for s in range(SPB):
            G[c, s, 0:RA] = c2[:, s * R:(s + 1) * R]
            G[c, s, RA:R] = c2[0:72, (2 + s) * R:(3 + s) * R]
    G = G.reshape(N, R, R)
    ms = np.concatenate(
        [np.concatenate([r["msa"], r["msb"]], axis=0)[None]
         for r in res.results], axis=0)           # (8, 200, SPB)
    ms = ms.transpose(0, 2, 1).reshape(N, R).astype(np.float64)  # (16, 200)
    G -= ms[:, :, None] * ms[:, None, :] / T
    d = np.einsum('nrr->nr', G)                   # ||C_r||^2
    rinv = 1.0 / (np.sqrt(d) + counts[None, :] * EPS)
    conn = G * rinv[:, :, None] * rinv[:, None, :]
    row, col = np.triu_indices(R, k=1)
    return np.ascontiguousarray(conn[:, row, col]).astype(np.float32)


# revision 15
# speedup vs baseline: 1.3442x; 1.3442x over previous
"""Connectome kernel (segment-mean -> Pearson Gram) for 8 TRN2 NeuronCores.

Strategy (pure data parallel, 2 samples per core):
  - Host marshalling: fold mask into parcellation; DROP background /
    masked-out pixels (~50% of V); sort survivors by ROI and pack into
    128-pixel chunks (block B = ROIs 128..199 first, then block A =
    ROIs 0..127; chunks padded with label -1 slots).
  - fp8 wire format with EXACT compensation: the whole computation
    depends on pixels only through per-ROI sums, so all pixels ship as
    fp8 e4m3 except ONE fp16 "compensator" pixel per ROI that carries
    its own value plus the summed fp8 quantization errors of its ROI.
    Per-ROI sums are therefore fp16-exact while the stream is ~1B/pixel
    (~9.3MB/core vs 18.3MB fp16, 73.7MB naive fp32). The compensator
    chunks are ROIs in order, so their onehots are the identity
    matrices already shipped as consts - no DVE build needed.
  - Device: stream fp8 chunk-tiles on the two HWDGE rings (byte-greedy
    ring assignment); batched DVE is_equal onehots (fp16 compare ->
    fp8 out); per chunk one PE matmul acc[r, row] += onehot.T @ x_chunk
    (fp8 operands, fp32 PSUM); fp16 identity matmuls for the two
    compensator chunks close each block's accumulation.
  - Centering cancels analytically: C C^T = S S^T - (1/T) m m^T, so
    the device Grams the RAW sums S (cast fp16) and ships tiny row
    sums; the host applies the rank-1 correction and 1/norm scaling.
    Block B's transposes overlap block A's stream; the tail is block
    A's cast + transposes, 8 Gram matmuls into 4 independent PSUM
    banks, four fp16 conn DMAs.
  - Host: concat cores, rank-1 correct, normalize, upper triangle
    -> (16, 19900) fp32.
"""
import sys

sys.path.insert(0, "/opt/trn_rl_repo")

import numpy as np

import concourse.bass as bass
import concourse.tile as tile
from concourse import bacc, mybir
from concourse.bass_utils import run_bass_kernel_spmd

F32 = mybir.dt.float32
F16 = mybir.dt.float16
F8 = mybir.dt.float8e4

N, T, H, W = 16, 200, 144, 320
V = H * W                      # 46080
R = 200                        # ROIs
RA = 128                       # ROI block A width (ROIs 0..127)
RB = R - RA                    # ROI block B width (72; ROIs 128..199)
NCORES = 8
SPB = N // NCORES              # samples per core = 2
ROWS = SPB * T                 # 400
EPS = 1e-8


def _f8(a):
    """Quantize to fp8 e4m3fn (returns ml_dtypes array)."""
    import ml_dtypes
    return a.astype(ml_dtypes.float8_e4m3fn)


def _tile_sizes(nch):
    """DMA tile schedule: small first tiles to fill the pipe fast, 8s in
    steady state, small tapered tiles at the end so the PE drain after
    the last transfer is short."""
    sizes, left = [], nch
    while left and len(sizes) < 4:
        ct = min(4, left)
        sizes.append(ct)
        left -= ct
    while left >= 16:
        sizes.append(8)
        left -= 8
    if left > 8:
        sizes.append(left - 8)
        left = 8
    while left:
        ct = min(4, left)
        sizes.append(ct)
        left -= ct
    return sizes


_cached = {}


def _bc3(ap2, ins_pos, n):
    """Insert a broadcast (stride 0, count n) dim into a 2D AP."""
    layout = [list(d) for d in ap2.ap]
    layout.insert(ins_pos, [0, n])
    return bass.AP(ap2.tensor, ap2.offset, layout)


def _split_st(ap2):
    """View a [P, SPB*T] AP as [P, SPB, T] (split the free dim)."""
    layout = [list(d) for d in ap2.ap]
    assert layout[-1][0] == 1 and layout[-1][1] == SPB * T
    layout = layout[:-1] + [[T, SPB], [1, T]]
    return bass.AP(ap2.tensor, ap2.offset, layout)


def _build_program(nA, nB):
    """nA/nB: fp8 chunk counts for ROI blocks A and B."""
    nch = nA + nB
    nc = bacc.Bacc("TRN2", target_bir_lowering=False, debug=False)

    # consts packed into one DRAM tensor: cols [0:nch] labs, [nch:+128]
    # iota, [+128:+256] i128, [+256:+328] i72 (partitions 72:128 zero).
    CC = nch + 328
    x_d = nc.declare_dram_parameter("x8", [128, nch, ROWS], F8, isOutput=False)
    xc_d = nc.declare_dram_parameter("xc", [128, 2, ROWS], F16, isOutput=False)
    cst_d = nc.declare_dram_parameter("consts", [128, CC], F16, isOutput=False)
    # conn2 cols: [0:200] G_s0 rois 0:128, [200:400] G_s1 rois 0:128,
    # [400:600] G_s0 rois 128:200 (parts 0:72), [600:800] G_s1 rois 128:200.
    out_d = nc.declare_dram_parameter("conn2", [128, 4 * R], F16, isOutput=True)
    msa_d = nc.declare_dram_parameter("msa", [RA, SPB], F32, isOutput=True)
    msb_d = nc.declare_dram_parameter("msb", [RB, SPB], F32, isOutput=True)

    tsizes = _tile_sizes(nch)
    # greedy byte-balanced ring assignment (sync starts with the consts,
    # scalar with the compensator chunks)
    ring_bytes = {0: CC * 2.0, 1: 2 * ROWS * 2.0}   # 0 = sync, 1 = scalar
    ring_of = []
    for ct in tsizes:
        r = 0 if ring_bytes[0] <= ring_bytes[1] else 1
        ring_of.append(r)
        ring_bytes[r] += ct * ROWS * 1.0            # fp8: 1 B/elem

    with tile.TileContext(nc) as tc:
        with tc.tile_pool(name="consts", bufs=1) as consts, \
             tc.tile_pool(name="loads", bufs=3) as loads, \
             tc.tile_pool(name="ohp", bufs=1) as ohp, \
             tc.tile_pool(name="epi", bufs=1) as epi, \
             tc.tile_pool(name="psum", bufs=1, space="PSUM") as psum:

            cst_s = consts.tile([128, CC], F16)
            # labs+iota (gates the first onehot build) land first; the
            # identities aren't needed until the compensator matmuls.
            nc.sync.dma_start(cst_s[:, 0:nch + 128], cst_d[:, 0:nch + 128])
            nc.sync.dma_start(cst_s[:, nch + 128:CC], cst_d[:, nch + 128:CC])
            labs_s = cst_s[:, 0:nch]
            iota_s = cst_s[:, nch:nch + 128]
            i128_s = cst_s[:, nch + 128:nch + 256]
            i72f_s = cst_s[:, nch + 256:nch + 328]   # [128, 72], rows 72+ zero
            i72_s = cst_s[0:72, nch + 256:nch + 328]
            ldc = consts.tile([128, 2, ROWS], F16)   # compensator chunks
            nc.scalar.dma_start(ldc[:], xc_d[:])

            acc_a = psum.tile([RA, ROWS], F32, tag="acc_a", bufs=1)
            acc_b = psum.tile([RB, ROWS], F32, tag="acc_b", bufs=1)

            # PSUM tr tiles: [t-block, roi] transposed raw-sum rows.
            tr = {}
            for s in range(SPB):
                tr[("A", s)] = psum.tile([128, R], F16, tag="trA", bufs=2,
                                         name=f"trA_{s}")
                tr[("B", s)] = psum.tile([72, R], F16, tag="trB", bufs=2,
                                         name=f"trB_{s}")

            def finish_block(blk, acc, P, ms_d, ms_eng):
                """Raw-sum epilogue for one ROI block: cast the PSUM sums
                to fp16 (Gram/transpose operand) and ship per-sample row
                sums (host applies the rank-1 centering correction)."""
                S16 = epi.tile([P, ROWS], F16, tag=f"S16_{blk}")
                ms = epi.tile([P, SPB], F32, tag=f"ms_{blk}")
                nc.vector.tensor_copy(S16[:], acc[:])
                nc.vector.tensor_reduce(ms[:], _split_st(acc[:]),
                                        axis=mybir.AxisListType.X,
                                        op=mybir.AluOpType.add)
                ms_eng.dma_start(ms_d[:], ms[:])
                return S16

            S16_b = None
            with nc.named_scope("main"):
                ch0 = 0
                for ti, ct in enumerate(tsizes):
                    ld = loads.tile([128, ct, ROWS], F8, tag=f"ld{ct}",
                                    bufs=(16 if ct == 8 else 4),
                                    name=f"ld_{ti}")
                    eng = nc.sync if ring_of[ti] == 0 else nc.scalar
                    eng.dma_start(ld[:], x_d[:, ch0:ch0 + ct, :])

                    # batched per-tile onehot builds (DVE, fp16 compare ->
                    # fp8 out), one per block segment present in this tile
                    nb_i = max(0, min(nB, ch0 + ct) - ch0)       # B chunks
                    na_i = ct - nb_i                             # A chunks
                    ohB_t = ohA_t = None
                    if nb_i:
                        ohB_t = ohp.tile([128, nb_i, RB], F8,
                                         tag=f"ohB{nb_i}", bufs=4,
                                         name=f"ohB_{ti}")
                        nc.vector.tensor_tensor(
                            ohB_t[:], _bc3(iota_s[:, 0:RB], 1, nb_i),
                            _bc3(labs_s[:, ch0:ch0 + nb_i], 2, RB),
                            op=mybir.AluOpType.is_equal)
                    if na_i:
                        a0 = ch0 + nb_i
                        ohA_t = ohp.tile([128, na_i, RA], F8,
                                         tag=f"ohA{na_i}", bufs=4,
                                         name=f"ohA_{ti}")
                        nc.vector.tensor_tensor(
                            ohA_t[:], _bc3(iota_s[:, 0:RA], 1, na_i),
                            _bc3(labs_s[:, a0:a0 + na_i], 2, RA),
                            op=mybir.AluOpType.is_equal)

                    for j in range(ct):
                        cc = ch0 + j
                        if cc < nB:
                            acc, oh = acc_b, ohB_t[:, j, :]
                            start = (cc == 0)
                        else:
                            acc, oh = acc_a, ohA_t[:, j - nb_i, :]
                            start = (cc == nB)
                        nc.tensor.matmul(acc[:], oh, ld[:, j, :],
                                         start=start, stop=False)
                        if cc == nB - 1:
                            # identity-weight fp16 compensator matmul
                            # closes block B.
                            nc.tensor.matmul(acc_b[:], i72f_s,
                                             ldc[:, 0, :],
                                             start=False, stop=True)
                    ch0 += ct

                    if ch0 - ct < nB <= ch0:
                        # block B complete: cast + row sums on DVE while
                        # block A still streams.
                        b_done_ti = ti
                        S16_b = finish_block("b", acc_b, RB, msb_d, nc.sync)
                    if S16_b is not None and ti == b_done_ti + 3:
                        # B-sourced transposes, emitted a few tiles later
                        # so the cast has finished and PE's FIFO never
                        # blocks on it.
                        for s in range(SPB):
                            nc.tensor.transpose(
                                tr[("A", s)][:, 128:200],
                                S16_b[:, s * T:s * T + 128], i72_s)
                            nc.tensor.transpose(
                                tr[("B", s)][:, 128:200],
                                S16_b[:, s * T + 128:s * T + 200], i72_s)
                # identity-weight fp16 compensator matmul closes block A.
                nc.tensor.matmul(acc_a[:], i128_s, ldc[:, 1, :],
                                 start=False, stop=True)

            with nc.named_scope("epilogue"):
                # block-A finish: casts split per sample so s0's transposes
                # start half a cast earlier; the row-sum reduce runs before
                # the Grams so acc_a's bank can be reused for Gram s1.
                S16_a = epi.tile([RA, ROWS], F16, tag="S16_a")
                ms_a = epi.tile([RA, SPB], F32, tag="ms_a")
                tr_sb = {}
                for s in range(SPB):
                    nc.vector.tensor_copy(S16_a[:, s * T:(s + 1) * T],
                                          acc_a[:, s * T:(s + 1) * T])
                    nc.tensor.transpose(tr[("A", s)][:, 0:128],
                                        S16_a[:, s * T:s * T + 128], i128_s)
                    nc.tensor.transpose(tr[("B", s)][:, 0:128],
                                        S16_a[:, s * T + 128:s * T + 200],
                                        i128_s)
                # row-sum reduce directly after the casts in the DVE queue:
                # it is acc_a's last reader, and sample 1's Grams reuse
                # that bank.
                nc.vector.tensor_reduce(ms_a[:], _split_st(acc_a[:]),
                                        axis=mybir.AxisListType.X,
                                        op=mybir.AluOpType.add)
                nc.sync.dma_start(msa_d[:], ms_a[:])
                for s in range(SPB):
                    trA_sb = epi.tile([128, R], F16, name=f"trAs_{s}",
                                      tag="trAs", bufs=2)
                    trB_sb = epi.tile([72, R], F16, name=f"trBs_{s}",
                                      tag="trBs", bufs=2)
                    nc.vector.tensor_copy(trA_sb[:], tr[("A", s)][:])
                    nc.vector.tensor_copy(trB_sb[:], tr[("B", s)][:])
                    tr_sb[s] = (trA_sb, trB_sb)

                # Gram: conn = S_t.T @ S_t (contraction over t, fp16);
                # four independent PSUM banks (sample 1 reuses the freed
                # acc_a/acc_b banks) so no Gram matmul ever waits on a
                # cast reading another sample's bank. cB DMAs ship all
                # 128 partitions (rows 72:128 are junk the host ignores)
                # - full-height transfers issue ~2x faster than 72-row.
                cA0 = psum.tile([128, R], F32, tag="cA1", name="cA0")
                cB0 = psum.tile([72, R], F32, tag="cB1", name="cB0")
                cgram = {("A", 0): cA0, ("B", 0): cB0,
                         ("A", 1): acc_a, ("B", 1): acc_b}
                connsb = epi.tile([128, 4 * R], F16, tag="connsb")
                for s in range(SPB):
                    trA_sb, trB_sb = tr_sb[s]
                    c = cgram[("A", s)]
                    nc.tensor.matmul(c[:, 0:R], trA_sb[:, 0:128], trA_sb[:],
                                     start=True, stop=False)
                    nc.tensor.matmul(c[:, 0:R], trB_sb[:, 0:128], trB_sb[:],
                                     start=False, stop=True)
                    nc.vector.tensor_copy(connsb[:, s * R:(s + 1) * R],
                                          c[:, 0:R])
                    nc.sync.dma_start(out_d[:, s * R:(s + 1) * R],
                                      connsb[:, s * R:(s + 1) * R])
                for s in range(SPB):
                    trA_sb, trB_sb = tr_sb[s]
                    c = cgram[("B", s)]
                    nc.tensor.matmul(c[:, 0:R], trA_sb[:, 128:200], trA_sb[:],
                                     start=True, stop=False)
                    nc.tensor.matmul(c[:, 0:R], trB_sb[:, 128:200], trB_sb[:],
                                     start=False, stop=True)
                    nc.vector.tensor_copy(connsb[0:72, (2 + s) * R:(3 + s) * R],
                                          c[:, 0:R])
                    nc.scalar.dma_start(out_d[:, (2 + s) * R:(3 + s) * R],
                                        connsb[:, (2 + s) * R:(3 + s) * R])

    nc.compile()
    return nc


def _get_program(nA, nB):
    key = (nA, nB)
    if key not in _cached:
        _cached[key] = _build_program(nA, nB)
    return _cached[key]


def marshal_inputs(x, parc, mask):
    """Host-side prep: ROI-sorted fp8 pixels + fp16 compensators whose
    values carry the summed fp8 quantization errors of their ROI, so
    per-ROI sums on device are fp16-exact."""
    import ml_dtypes

    parc_eff = np.where(np.asarray(mask), np.asarray(parc), 0).reshape(V)
    lab = parc_eff.astype(np.int64) - 1          # -1 = dropped
    counts = np.bincount(parc_eff.astype(np.int64), minlength=R + 1)[1:]
    assert counts.min() >= 2, "compensator scheme needs >=2 pixels per ROI"

    order = np.argsort(lab, kind="stable")
    nbg = int((lab < 0).sum())
    sorted_idx = order[nbg:]                     # kept pixels, ROI-ascending
    K = sorted_idx.size
    labk = lab[sorted_idx]
    bounds = np.searchsorted(labk, np.arange(R))  # ROI start offsets
    comp_pos = bounds                             # first pixel of each ROI
    rest_mask = np.ones(K, bool)
    rest_mask[comp_pos] = False
    rest_sorted = sorted_idx[rest_mask]          # ROI-sorted non-compensators
    lab_rest = labk[rest_mask]
    rb = np.searchsorted(lab_rest, np.arange(R))  # rest ROI start offsets
    comp_idx = sorted_idx[comp_pos]              # (R,) pixel index per ROI

    cB8 = int((lab_rest >= RA).sum())
    cA8 = int((lab_rest < RA).sum())
    nB = (cB8 + 127) // 128
    nA = (cA8 + 127) // 128
    nch = nA + nB

    restB = rest_sorted[lab_rest >= RA]
    restA = rest_sorted[lab_rest < RA]
    labB = np.concatenate([lab_rest[lab_rest >= RA] - RA,
                           np.full(nB * 128 - cB8, -1, dtype=np.int64)])
    labA = np.concatenate([lab_rest[lab_rest < RA],
                           np.full(nA * 128 - cA8, -1, dtype=np.int64)])
    labs = np.concatenate([labB, labA]).astype(np.float16)
    labs = labs.reshape(nch, 128).T.copy()       # (128, nch)

    iota = np.broadcast_to(np.arange(128, dtype=np.float16),
                           (128, 128)).copy()    # iota[p, c] = c
    i128 = np.eye(128, dtype=np.float16)
    i72 = np.zeros((128, 72), dtype=np.float16)
    i72[:72] = np.eye(72, dtype=np.float16)
    consts = np.concatenate([labs, iota, i128, i72], axis=1)  # (128, nch+328)

    # quantize + compensate per sample (bounds transient memory)
    x32 = np.asarray(x, dtype=np.float32).reshape(N, T, V)
    xq8 = np.empty((N, T, nch * 128), dtype=ml_dtypes.float8_e4m3fn)
    xq8[:, :, cB8:nB * 128] = 0
    xq8[:, :, nB * 128 + cA8:] = 0
    ycomp = np.empty((N, T, R), np.float16)
    for n in range(N):
        xr = x32[n][:, rest_sorted]              # (T, K-R) ROI-sorted
        q = xr.astype(ml_dtypes.float8_e4m3fn)
        e = xr.astype(np.float64) - q.astype(np.float64)
        esum = np.add.reduceat(e, rb, axis=1)    # (T, R) per-ROI error sums
        ycomp[n] = (x32[n][:, comp_idx].astype(np.float64) + esum
                    ).astype(np.float16)
        qB = q[:, lab_rest >= RA]
        qA = q[:, lab_rest < RA]
        xq8[n, :, 0:cB8] = qB
        xq8[n, :, nB * 128:nB * 128 + cA8] = qA

    # (N, T, nch*128) fp8 -> packed (core, 128, nch, SPB*T)
    xg = xq8.reshape(NCORES, SPB, T, nch, 128)
    xs = np.ascontiguousarray(xg.transpose(0, 4, 3, 1, 2))  # (8,128,nch,2,T)
    xs = xs.reshape(NCORES, 128, nch, ROWS)

    # compensator chunks (N, T, 2, 128): chunk 0 = B comps, chunk 1 = A
    xc = np.zeros((N, T, 2, 128), np.float16)
    xc[:, :, 0, 0:RB] = ycomp[:, :, RA:R]
    xc[:, :, 1, :] = ycomp[:, :, 0:RA]
    xcg = xc.reshape(NCORES, SPB, T, 2, 128)
    xcs = np.ascontiguousarray(xcg.transpose(0, 4, 3, 1, 2))  # (8,128,2,2,T)
    xcs = xcs.reshape(NCORES, 128, 2, ROWS)

    in_maps = []
    for c in range(NCORES):
        in_maps.append({"x8": xs[c], "xc": xcs[c], "consts": consts})
    return in_maps, nA, nB, counts


def kernel(x, parc, mask):
    in_maps, nA, nB, counts = marshal_inputs(x, parc, mask)
    nc = _get_program(nA, nB)
    res = run_bass_kernel_spmd(nc, in_maps, core_ids=list(range(NCORES)))
    # device emits the raw-sum Gram (fp16) + per-sample row sums; the
    # centering is a host-side rank-1 correction (C C^T = S S^T - m m^T/T
    # with m = row sums), and normalization a rank-1 scaling.
    G = np.empty((NCORES, SPB, R, R), np.float64)
    for c, r in enumerate(res.results):
        c2 = r["conn2"].astype(np.float64)       # (128, 800)
        for s in range(SPB):
            G[c, s, 0:RA] = c2[:, s * R:(s + 1) * R]
            G[c, s, RA:R] = c2[0:72, (2 + s) * R:(3 + s) * R]
    G = G.reshape(N, R, R)
    ms = np.concatenate(
        [np.concatenate([r["msa"], r["msb"]], axis=0)[None]
         for r in res.results], axis=0)           # (8, 200, SPB)
    ms = ms.transpose(0, 2, 1).reshape(N, R).astype(np.float64)  # (16, 200)
    G -= ms[:, :, None] * ms[:, None, :] / T
    d = np.einsum('nrr->nr', G)                   # ||C_r||^2
    rinv = 1.0 / (np.sqrt(d) + counts[None, :] * EPS)
    conn = G * rinv[:, :, None] * rinv[:, None, :]
    row, col = np.triu_indices(R, k=1)
    return np.ascontiguousarray(conn[:, row, col]).astype(np.float32)


# revision 20
# speedup vs baseline: 1.4789x; 1.1002x over previous
"""Connectome kernel (segment-mean -> Pearson Gram) for 8 TRN2 NeuronCores.

Strategy (pure data parallel, 2 samples per core):
  - Host marshalling: fold mask into parcellation; DROP background /
    masked-out pixels (~50% of V); sort survivors by ROI and pack into
    128-pixel chunks (block B = ROIs 128..199 first, then block A =
    ROIs 0..127; chunks padded with label -1 slots).
  - fp8 wire format with EXACT compensation: the whole computation
    depends on pixels only through per-ROI sums, so all pixels ship as
    fp8 e4m3 except ONE fp16 "compensator" pixel per ROI that carries
    its own value plus the summed fp8 quantization errors of its ROI.
    Per-ROI sums are therefore fp16-exact while the stream is ~1B/pixel
    (~9.3MB/core vs 18.3MB fp16, 73.7MB naive fp32). The compensator
    chunks are ROIs in order, so their onehots are the identity
    matrices already shipped as consts - no DVE build needed.
  - Device: stream fp8 chunk-tiles on the two HWDGE rings (byte-greedy
    ring assignment); batched DVE is_equal onehots (fp16 compare ->
    fp8 out); per chunk one PE matmul acc[r, row] += onehot.T @ x_chunk
    (fp8 operands, fp32 PSUM); fp16 identity matmuls for the two
    compensator chunks close each block's accumulation.
  - Centering cancels analytically: C C^T = S S^T - (1/T) m m^T, so
    the device Grams the RAW sums S (cast fp16) and ships tiny row
    sums; the host applies the rank-1 correction and 1/norm scaling.
    Block B's transposes overlap block A's stream; the tail is block
    A's cast + transposes, 8 Gram matmuls into 4 independent PSUM
    banks, four fp16 conn DMAs.
  - Host: concat cores, rank-1 correct, normalize, upper triangle
    -> (16, 19900) fp32.
"""
import sys

sys.path.insert(0, "/opt/trn_rl_repo")

import numpy as np

import concourse.bass as bass
import concourse.tile as tile
from concourse import bacc, mybir
from concourse.bass_utils import run_bass_kernel_spmd

F32 = mybir.dt.float32
F16 = mybir.dt.float16
F8 = mybir.dt.float8e4

N, T, H, W = 16, 200, 144, 320
V = H * W                      # 46080
R = 200                        # ROIs
RA = 128                       # ROI block A width (ROIs 0..127)
RB = R - RA                    # ROI block B width (72; ROIs 128..199)
NCORES = 8
SPB = N // NCORES              # samples per core = 2
ROWS = SPB * T                 # 400
EPS = 1e-8


def _f8(a):
    """Quantize to fp8 e4m3fn (returns ml_dtypes array)."""
    import ml_dtypes
    return a.astype(ml_dtypes.float8_e4m3fn)


def _tile_sizes(nch):
    """DMA tile schedule: small first tiles to fill the pipe fast, 8s in
    steady state, small tapered tiles at the end so the PE drain after
    the last transfer is short."""
    sizes, left = [], nch
    while left and len(sizes) < 4:
        ct = min(4, left)
        sizes.append(ct)
        left -= ct
    while left >= 16:
        sizes.append(8)
        left -= 8
    if left > 8:
        sizes.append(left - 8)
        left = 8
    while left:
        ct = min(4, left)
        sizes.append(ct)
        left -= ct
    return sizes


_cached = {}


def _bc3(ap2, ins_pos, n):
    """Insert a broadcast (stride 0, count n) dim into a 2D AP."""
    layout = [list(d) for d in ap2.ap]
    layout.insert(ins_pos, [0, n])
    return bass.AP(ap2.tensor, ap2.offset, layout)


def _split_st(ap2):
    """View a [P, SPB*T] AP as [P, SPB, T] (split the free dim)."""
    layout = [list(d) for d in ap2.ap]
    assert layout[-1][0] == 1 and layout[-1][1] == SPB * T
    layout = layout[:-1] + [[T, SPB], [1, T]]
    return bass.AP(ap2.tensor, ap2.offset, layout)


def _build_program(nA, nB):
    """nA/nB: fp8 chunk counts for ROI blocks A and B."""
    nch = nA + nB
    nc = bacc.Bacc("TRN2", target_bir_lowering=False, debug=False)

    # consts packed into one DRAM tensor: cols [0:nch] labs, [nch:+128]
    # iota, [+128:+256] i128, [+256:+328] i72 (partitions 72:128 zero).
    CC = nch + 328
    x_d = nc.declare_dram_parameter("x8", [128, nch, ROWS], F8, isOutput=False)
    xc_d = nc.declare_dram_parameter("xc", [128, 2, ROWS], F16, isOutput=False)
    cst_d = nc.declare_dram_parameter("consts", [128, CC], F16, isOutput=False)
    # conn2 cols: [0:200] G_s0 rois 0:128, [200:400] G_s1 rois 0:128,
    # [400:600] G_s0 rois 128:200 (parts 0:72), [600:800] G_s1 rois 128:200.
    out_d = nc.declare_dram_parameter("conn2", [128, 4 * R], F16, isOutput=True)
    msa_d = nc.declare_dram_parameter("msa", [RA, SPB], F32, isOutput=True)
    msb_d = nc.declare_dram_parameter("msb", [RB, SPB], F32, isOutput=True)

    tsizes = _tile_sizes(nch)
    # greedy byte-balanced ring assignment: ALL consts + compensators go
    # on scalar so sync streams x from the first cycle (small-descriptor
    # consts transfers would otherwise delay the first tiles ~4us).
    ring_bytes = {0: 0.0, 1: CC * 2.0 + 2 * ROWS * 2.0}  # 0 = sync, 1 = scalar
    ring_of = []
    for ct in tsizes:
        r = 0 if ring_bytes[0] <= ring_bytes[1] else 1
        ring_of.append(r)
        ring_bytes[r] += ct * ROWS * 1.0            # fp8: 1 B/elem

    with tile.TileContext(nc) as tc:
        with tc.tile_pool(name="consts", bufs=1) as consts, \
             tc.tile_pool(name="loads", bufs=3) as loads, \
             tc.tile_pool(name="ohp", bufs=1) as ohp, \
             tc.tile_pool(name="epi", bufs=1) as epi, \
             tc.tile_pool(name="psum", bufs=1, space="PSUM") as psum:

            cst_s = consts.tile([128, CC], F16)
            # labs+iota (gates the first onehot build) land first; the
            # identities aren't needed until the compensator matmuls.
            nc.scalar.dma_start(cst_s[:, 0:nch + 128], cst_d[:, 0:nch + 128])
            nc.scalar.dma_start(cst_s[:, nch + 128:CC], cst_d[:, nch + 128:CC])
            labs_s = cst_s[:, 0:nch]
            iota_s = cst_s[:, nch:nch + 128]
            i128_s = cst_s[:, nch + 128:nch + 256]
            i72f_s = cst_s[:, nch + 256:nch + 328]   # [128, 72], rows 72+ zero
            i72_s = cst_s[0:72, nch + 256:nch + 328]
            ldc = consts.tile([128, 2, ROWS], F16)   # compensator chunks
            nc.scalar.dma_start(ldc[:], xc_d[:])

            # acc_b is 80 partitions: block-B onehots pad 72->80 so the
            # DoubleRow weight k-tile step (80 fp8 bytes) is 16B-aligned;
            # partitions 72:80 only ever accumulate zeros.
            RB8 = 80
            acc_a = psum.tile([RA, ROWS], F32, tag="acc_a", bufs=1)
            acc_b = psum.tile([RB8, ROWS], F32, tag="acc_b", bufs=1)

            # PSUM tr tiles: [t-block, roi] transposed raw-sum rows.
            tr = {}
            for s in range(SPB):
                tr[("A", s)] = psum.tile([128, R], F16, tag="trA", bufs=2,
                                         name=f"trA_{s}")
                tr[("B", s)] = psum.tile([72, R], F16, tag="trB", bufs=2,
                                         name=f"trB_{s}")

            def finish_block(blk, acc_ap, P, ms_d, ms_eng):
                """Raw-sum epilogue for one ROI block: cast the PSUM sums
                to fp16 (Gram/transpose operand) and ship per-sample row
                sums (host applies the rank-1 centering correction)."""
                S16 = epi.tile([P, ROWS], F16, tag=f"S16_{blk}")
                ms = epi.tile([P, SPB], F32, tag=f"ms_{blk}")
                nc.vector.tensor_copy(S16[:], acc_ap)
                nc.vector.tensor_reduce(ms[:], _split_st(acc_ap),
                                        axis=mybir.AxisListType.X,
                                        op=mybir.AluOpType.add)
                ms_eng.dma_start(ms_d[:], ms[:])
                return S16

            S16_b = None
            with nc.named_scope("main"):
                ch0 = 0
                for ti, ct in enumerate(tsizes):
                    ld = loads.tile([128, ct, ROWS], F8, tag=f"ld{ct}",
                                    bufs=(16 if ct == 8 else 4),
                                    name=f"ld_{ti}")
                    eng = nc.sync if ring_of[ti] == 0 else nc.scalar
                    eng.dma_start(ld[:], x_d[:, ch0:ch0 + ct, :])

                    # batched per-tile onehot builds (DVE, fp16 compare ->
                    # fp8 out), one per block segment present in this tile
                    nb_i = max(0, min(nB, ch0 + ct) - ch0)       # B chunks
                    na_i = ct - nb_i                             # A chunks
                    ohB_t = ohA_t = None
                    if nb_i:
                        ohB_t = ohp.tile([128, nb_i, RB8], F8,
                                         tag=f"ohB{nb_i}", bufs=4,
                                         name=f"ohB_{ti}")
                        nc.vector.tensor_tensor(
                            ohB_t[:], _bc3(iota_s[:, 0:RB8], 1, nb_i),
                            _bc3(labs_s[:, ch0:ch0 + nb_i], 2, RB8),
                            op=mybir.AluOpType.is_equal)
                    if na_i:
                        a0 = ch0 + nb_i
                        ohA_t = ohp.tile([128, na_i, RA], F8,
                                         tag=f"ohA{na_i}", bufs=4,
                                         name=f"ohA_{ti}")
                        nc.vector.tensor_tensor(
                            ohA_t[:], _bc3(iota_s[:, 0:RA], 1, na_i),
                            _bc3(labs_s[:, a0:a0 + na_i], 2, RA),
                            op=mybir.AluOpType.is_equal)

                    # chunk-pair DoubleRow matmuls (2 chunks = 256
                    # contraction rows per instruction); odd segment
                    # leftovers fall back to single normal-mode matmuls.
                    for seg0, seg1, acc, oh_t, j_off in (
                            (ch0, ch0 + nb_i, acc_b, ohB_t, 0),
                            (ch0 + nb_i, ch0 + ct, acc_a, ohA_t, nb_i)):
                        j = seg0
                        while j < seg1:
                            start = (j == 0) if acc is acc_b else (j == nB)
                            jl = j - ch0          # tile-local chunk index
                            ol = j - ch0 - j_off  # oh-tile-local index
                            if j + 1 < seg1:
                                nc.tensor.matmul(
                                    acc[:], oh_t[:, ol:ol + 2, :],
                                    ld[:, jl:jl + 2, :],
                                    start=start, stop=False,
                                    perf_mode=mybir.MatmulPerfMode.DoubleRow)
                                j += 2
                            else:
                                nc.tensor.matmul(acc[:], oh_t[:, ol, :],
                                                 ld[:, jl, :],
                                                 start=start, stop=False)
                                j += 1
                        if seg0 <= nB - 1 < seg1:
                            # identity-weight fp16 compensator matmul
                            # closes block B.
                            nc.tensor.matmul(acc_b[0:72, :], i72f_s,
                                             ldc[:, 0, :],
                                             start=False, stop=True)
                    ch0 += ct

                    if ch0 - ct < nB <= ch0:
                        # block B complete: cast + row sums on DVE while
                        # block A still streams.
                        b_done_ti = ti
                        S16_b = finish_block("b", acc_b[0:72, :], RB,
                                             msb_d, nc.sync)
                    if S16_b is not None and ti == b_done_ti + 3:
                        # B-sourced transposes, emitted a few tiles later
                        # so the cast has finished and PE's FIFO never
                        # blocks on it.
                        for s in range(SPB):
                            nc.tensor.transpose(
                                tr[("A", s)][:, 128:200],
                                S16_b[:, s * T:s * T + 128], i72_s)
                            nc.tensor.transpose(
                                tr[("B", s)][:, 128:200],
                                S16_b[:, s * T + 128:s * T + 200], i72_s)
                # identity-weight fp16 compensator matmul closes block A.
                nc.tensor.matmul(acc_a[:], i128_s, ldc[:, 1, :],
                                 start=False, stop=True)

            with nc.named_scope("epilogue"):
                # block-A finish: casts split per sample so s0's transposes
                # start half a cast earlier; the row-sum reduce runs before
                # the Grams so acc_a's bank can be reused for Gram s1.
                S16_a = epi.tile([RA, ROWS], F16, tag="S16_a")
                ms_a = epi.tile([RA, SPB], F32, tag="ms_a")
                tr_sb = {}
                for s in range(SPB):
                    nc.vector.tensor_copy(S16_a[:, s * T:(s + 1) * T],
                                          acc_a[:, s * T:(s + 1) * T])
                    nc.tensor.transpose(tr[("A", s)][:, 0:128],
                                        S16_a[:, s * T:s * T + 128], i128_s)
                    nc.tensor.transpose(tr[("B", s)][:, 0:128],
                                        S16_a[:, s * T + 128:s * T + 200],
                                        i128_s)
                # row-sum reduce directly after the casts in the DVE queue:
                # it is acc_a's last reader, and sample 1's Grams reuse
                # that bank.
                nc.vector.tensor_reduce(ms_a[:], _split_st(acc_a[:]),
                                        axis=mybir.AxisListType.X,
                                        op=mybir.AluOpType.add)
                nc.sync.dma_start(msa_d[:], ms_a[:])
                for s in range(SPB):
                    trA_sb = epi.tile([128, R], F16, name=f"trAs_{s}",
                                      tag="trAs", bufs=2)
                    trB_sb = epi.tile([72, R], F16, name=f"trBs_{s}",
                                      tag="trBs", bufs=2)
                    nc.vector.tensor_copy(trA_sb[:], tr[("A", s)][:])
                    nc.vector.tensor_copy(trB_sb[:], tr[("B", s)][:])
                    tr_sb[s] = (trA_sb, trB_sb)

                # Gram: conn = S_t.T @ S_t (contraction over t, fp16);
                # four independent PSUM banks (sample 1 reuses the freed
                # acc_a/acc_b banks) so no Gram matmul ever waits on a
                # cast reading another sample's bank. cB DMAs ship all
                # 128 partitions (rows 72:128 are junk the host ignores)
                # - full-height transfers issue ~2x faster than 72-row.
                cA0 = psum.tile([128, R], F32, tag="cA1", name="cA0")
                cB0 = psum.tile([72, R], F32, tag="cB1", name="cB0")
                cgram = {("A", 0): cA0[:], ("B", 0): cB0[:],
                         ("A", 1): acc_a[:, 0:R], ("B", 1): acc_b[0:72, 0:R]}
                connsb = epi.tile([128, 4 * R], F16, tag="connsb")
                for s in range(SPB):
                    trA_sb, trB_sb = tr_sb[s]
                    c = cgram[("A", s)]
                    nc.tensor.matmul(c, trA_sb[:, 0:128], trA_sb[:],
                                     start=True, stop=False)
                    nc.tensor.matmul(c, trB_sb[:, 0:128], trB_sb[:],
                                     start=False, stop=True)
                    nc.vector.tensor_copy(connsb[:, s * R:(s + 1) * R], c)
                    nc.sync.dma_start(out_d[:, s * R:(s + 1) * R],
                                      connsb[:, s * R:(s + 1) * R])
                for s in range(SPB):
                    trA_sb, trB_sb = tr_sb[s]
                    c = cgram[("B", s)]
                    nc.tensor.matmul(c, trA_sb[:, 128:200], trA_sb[:],
                                     start=True, stop=False)
                    nc.tensor.matmul(c, trB_sb[:, 128:200], trB_sb[:],
                                     start=False, stop=True)
                    nc.vector.tensor_copy(connsb[0:72, (2 + s) * R:(3 + s) * R],
                                          c)
                    nc.scalar.dma_start(out_d[:, (2 + s) * R:(3 + s) * R],
                                        connsb[:, (2 + s) * R:(3 + s) * R])

    nc.compile()
    return nc


def _get_program(nA, nB):
    key = (nA, nB)
    if key not in _cached:
        _cached[key] = _build_program(nA, nB)
    return _cached[key]


def marshal_inputs(x, parc, mask):
    """Host-side prep: ROI-sorted fp8 pixels + fp16 compensators whose
    values carry the summed fp8 quantization errors of their ROI, so
    per-ROI sums on device are fp16-exact."""
    import ml_dtypes

    parc_eff = np.where(np.asarray(mask), np.asarray(parc), 0).reshape(V)
    lab = parc_eff.astype(np.int64) - 1          # -1 = dropped
    counts = np.bincount(parc_eff.astype(np.int64), minlength=R + 1)[1:]
    assert counts.min() >= 2, "compensator scheme needs >=2 pixels per ROI"

    order = np.argsort(lab, kind="stable")
    nbg = int((lab < 0).sum())
    sorted_idx = order[nbg:]                     # kept pixels, ROI-ascending
    K = sorted_idx.size
    labk = lab[sorted_idx]
    bounds = np.searchsorted(labk, np.arange(R))  # ROI start offsets
    comp_pos = bounds                             # first pixel of each ROI
    rest_mask = np.ones(K, bool)
    rest_mask[comp_pos] = False
    rest_sorted = sorted_idx[rest_mask]          # ROI-sorted non-compensators
    lab_rest = labk[rest_mask]
    rb = np.searchsorted(lab_rest, np.arange(R))  # rest ROI start offsets
    comp_idx = sorted_idx[comp_pos]              # (R,) pixel index per ROI

    cB8 = int((lab_rest >= RA).sum())
    cA8 = int((lab_rest < RA).sum())
    nB = (cB8 + 127) // 128
    nA = (cA8 + 127) // 128
    nch = nA + nB

    restB = rest_sorted[lab_rest >= RA]
    restA = rest_sorted[lab_rest < RA]
    labB = np.concatenate([lab_rest[lab_rest >= RA] - RA,
                           np.full(nB * 128 - cB8, -1, dtype=np.int64)])
    labA = np.concatenate([lab_rest[lab_rest < RA],
                           np.full(nA * 128 - cA8, -1, dtype=np.int64)])
    labs = np.concatenate([labB, labA]).astype(np.float16)
    labs = labs.reshape(nch, 128).T.copy()       # (128, nch)

    iota = np.broadcast_to(np.arange(128, dtype=np.float16),
                           (128, 128)).copy()    # iota[p, c] = c
    i128 = np.eye(128, dtype=np.float16)
    i72 = np.zeros((128, 72), dtype=np.float16)
    i72[:72] = np.eye(72, dtype=np.float16)
    consts = np.concatenate([labs, iota, i128, i72], axis=1)  # (128, nch+328)

    # quantize + compensate per sample (bounds transient memory)
    x32 = np.asarray(x, dtype=np.float32).reshape(N, T, V)
    xq8 = np.empty((N, T, nch * 128), dtype=ml_dtypes.float8_e4m3fn)
    xq8[:, :, cB8:nB * 128] = 0
    xq8[:, :, nB * 128 + cA8:] = 0
    ycomp = np.empty((N, T, R), np.float16)
    for n in range(N):
        xr = x32[n][:, rest_sorted]              # (T, K-R) ROI-sorted
        q = xr.astype(ml_dtypes.float8_e4m3fn)
        e = xr.astype(np.float64) - q.astype(np.float64)
        esum = np.add.reduceat(e, rb, axis=1)    # (T, R) per-ROI error sums
        ycomp[n] = (x32[n][:, comp_idx].astype(np.float64) + esum
                    ).astype(np.float16)
        qB = q[:, lab_rest >= RA]
        qA = q[:, lab_rest < RA]
        xq8[n, :, 0:cB8] = qB
        xq8[n, :, nB * 128:nB * 128 + cA8] = qA

    # (N, T, nch*128) fp8 -> packed (core, 128, nch, SPB*T)
    xg = xq8.reshape(NCORES, SPB, T, nch, 128)
    xs = np.ascontiguousarray(xg.transpose(0, 4, 3, 1, 2))  # (8,128,nch,2,T)
    xs = xs.reshape(NCORES, 128, nch, ROWS)

    # compensator chunks (N, T, 2, 128): chunk 0 = B comps, chunk 1 = A
    xc = np.zeros((N, T, 2, 128), np.float16)
    xc[:, :, 0, 0:RB] = ycomp[:, :, RA:R]
    xc[:, :, 1, :] = ycomp[:, :, 0:RA]
    xcg = xc.reshape(NCORES, SPB, T, 2, 128)
    xcs = np.ascontiguousarray(xcg.transpose(0, 4, 3, 1, 2))  # (8,128,2,2,T)
    xcs = xcs.reshape(NCORES, 128, 2, ROWS)

    in_maps = []
    for c in range(NCORES):
        in_maps.append({"x8": xs[c], "xc": xcs[c], "consts": consts})
    return in_maps, nA, nB, counts


def kernel(x, parc, mask):
    in_maps, nA, nB, counts = marshal_inputs(x, parc, mask)
    nc = _get_program(nA, nB)
    res = run_bass_kernel_spmd(nc, in_maps, core_ids=list(range(NCORES)))
    # device emits the raw-sum Gram (fp16) + per-sample row sums; the
    # centering is a host-side rank-1 correction (C C^T = S S^T - m m^T/T
    # with m = row sums), and normalization a rank-1 scaling.
    G = np.empty((NCORES, SPB, R, R), np.float64)
    for c, r in enumerate(res.results):
        c2 = r["conn2"].astype(np.float64)       # (128, 800)
        for s in range(SPB):
            G[c, s, 0:RA] = c2[:, s * R:(s + 1) * R]
            G[c, s, RA:R] = c2[0:72, (2 + s) * R:(3 + s) * R]
    G = G.reshape(N, R, R)
    ms = np.concatenate(
        [np.concatenate([r["msa"], r["msb"]], axis=0)[None]
         for r in res.results], axis=0)           # (8, 200, SPB)
    ms = ms.transpose(0, 2, 1).reshape(N, R).astype(np.float64)  # (16, 200)
    G -= ms[:, :, None] * ms[:, None, :] / T
    d = np.einsum('nrr->nr', G)                   # ||C_r||^2
    rinv = 1.0 / (np.sqrt(d) + counts[None, :] * EPS)
    conn = G * rinv[:, :, None] * rinv[:, None, :]
    row, col = np.triu_indices(R, k=1)
    return np.ascontiguousarray(conn[:, row, col]).astype(np.float32)


# revision 24
# speedup vs baseline: 1.5089x; 1.0203x over previous
"""Connectome kernel (segment-mean -> Pearson Gram) for 8 TRN2 NeuronCores.

Strategy (pure data parallel, 2 samples per core):
  - Host marshalling: fold mask into parcellation; DROP background /
    masked-out pixels (~50% of V); sort survivors by ROI and pack into
    128-pixel chunks, grouped into FOUR ROI blocks of width <=64
    (rois [0:64) [64:128) [128:192) [192:200)), streamed in descending
    block order so three blocks finish mid-stream and only block 0's
    epilogue is tail work. Narrow blocks halve the DVE onehot work
    (onehot elems = pixels x block width).
  - fp8 wire format with EXACT compensation: the whole computation
    depends on pixels only through per-ROI sums, so all pixels ship as
    fp8 e4m3 except ONE fp16 "compensator" pixel per ROI that carries
    its own value plus the summed fp8 quantization errors of its ROI.
    Per-ROI sums are therefore fp16-exact while the stream is ~1B/pixel
    (~9.3MB/core vs 18.3MB fp16, 73.7MB naive fp32). Compensator
    chunks hold blocks' ROIs in order, so their matmul weights are
    identity-matrix column slices already shipped as consts.
  - Device: stream fp8 chunk-tiles on the two HWDGE rings (byte-greedy
    ring assignment; all consts on scalar so sync streams x first);
    batched DVE is_equal onehots (fp16 compare -> fp8 out); chunk-PAIR
    DoubleRow fp8 matmuls (256 contraction rows per instruction)
    accumulate acc[r, row] += onehot.T @ x_chunk in fp32 PSUM; fp16
    identity matmuls for the compensator chunks close each block.
  - Centering cancels analytically: C C^T = S S^T - (1/T) m m^T, so
    the device Grams the RAW sums S (cast fp16) and ships tiny row
    sums; the host applies the rank-1 correction and 1/norm scaling.
  - Host: concat cores, rank-1 correct, normalize, upper triangle
    -> (16, 19900) fp32.
"""
import sys

sys.path.insert(0, "/opt/trn_rl_repo")

import numpy as np

import concourse.bass as bass
import concourse.tile as tile
from concourse import bacc, mybir
from concourse.bass_utils import run_bass_kernel_spmd

F32 = mybir.dt.float32
F16 = mybir.dt.float16
F8 = mybir.dt.float8e4

N, T, H, W = 16, 200, 144, 320
V = H * W                      # 46080
R = 200                        # ROIs
NCORES = 8
SPB = N // NCORES              # samples per core = 2
ROWS = SPB * T                 # 400
EPS = 1e-8

NBLK = 3
BW = 64                        # ROI block stride
BLK_W = [64, 64, 72]           # widths per block (rois 64k..64k+W)
BLK_WP = [64, 64, 80]          # padded onehot/acc widths (16B DoubleRow step)
SORDER = [2, 1, 0]             # stream order: block 0 last (tail block)


def _tile_sizes(nch):
    """DMA tile schedule: small first tiles to fill the pipe fast, 8s in
    steady state, small tapered tiles at the end so the PE drain after
    the last transfer is short."""
    sizes, left = [], nch
    while left and len(sizes) < 4:
        ct = min(4, left)
        sizes.append(ct)
        left -= ct
    while left >= 16:
        sizes.append(8)
        left -= 8
    if left > 8:
        sizes.append(left - 8)
        left = 8
    while left:
        ct = min(4, left)
        sizes.append(ct)
        left -= ct
    return sizes


_cached = {}


def _bc3(ap2, ins_pos, n):
    """Insert a broadcast (stride 0, count n) dim into a 2D AP."""
    layout = [list(d) for d in ap2.ap]
    layout.insert(ins_pos, [0, n])
    return bass.AP(ap2.tensor, ap2.offset, layout)


def _split_st(ap2):
    """View a [P, SPB*T] AP as [P, SPB, T] (split the free dim)."""
    layout = [list(d) for d in ap2.ap]
    assert layout[-1][0] == 1 and layout[-1][1] == SPB * T
    layout = layout[:-1] + [[T, SPB], [1, T]]
    return bass.AP(ap2.tensor, ap2.offset, layout)


def _build_program(ns):
    """ns: fp8 chunk counts per ROI block (index = block id 0..3)."""
    nch = sum(ns)
    nc = bacc.Bacc("TRN2", target_bir_lowering=False, debug=False)

    # consts cols: [0:nch] labs, then iota(128), i128(128), i64(64), i72(72)
    CC = nch + 392
    x_d = nc.declare_dram_parameter("x8", [128, nch, ROWS], F8, isOutput=False)
    xc_d = nc.declare_dram_parameter("xc", [128, 2, ROWS], F16, isOutput=False)
    cst_d = nc.declare_dram_parameter("consts", [128, CC], F16, isOutput=False)
    # conn2 cols: [0:200] G_s0 rois 0:128, [200:400] G_s1 rois 0:128,
    # [400:600] G_s0 rois 128:200 (parts 0:72), [600:800] G_s1 rois 128:200.
    out_d = nc.declare_dram_parameter("conn2", [128, 4 * R], F16, isOutput=True)
    ms_d = [nc.declare_dram_parameter(f"ms{k}", [BLK_W[k], SPB], F32,
                                      isOutput=True) for k in range(NBLK)]

    # stream-ordered block table: chunk ranges in the global chunk index
    blk_of_chunk = []
    chunk0 = {}
    for k in SORDER:
        chunk0[k] = len(blk_of_chunk)
        blk_of_chunk += [k] * ns[k]
    last_chunk = {k: chunk0[k] + ns[k] - 1 for k in SORDER}
    # compensator chunk + identity-weight column slice per block:
    # ldc chunk 0 = [blk2 comps (72) | zeros], chunk 1 =
    # [blk1 comps (64) | blk0 comps (64)].
    comp_map = {2: (0, 0, 72), 1: (1, 0, 64), 0: (1, 64, 128)}

    tsizes = _tile_sizes(nch)
    # greedy byte-balanced ring assignment: ALL consts + compensators go
    # on scalar so sync streams x from the first cycle (small-descriptor
    # consts transfers would otherwise delay the first tiles ~4us).
    ring_bytes = {0: 0.0, 1: CC * 2.0 + 2 * ROWS * 2.0}  # 0 = sync, 1 = scalar
    ring_of = []
    for ct in tsizes:
        r = 0 if ring_bytes[0] <= ring_bytes[1] else 1
        ring_of.append(r)
        ring_bytes[r] += ct * ROWS * 1.0            # fp8: 1 B/elem

    with tile.TileContext(nc) as tc:
        with tc.tile_pool(name="consts", bufs=1) as consts, \
             tc.tile_pool(name="loads", bufs=3) as loads, \
             tc.tile_pool(name="ohp", bufs=1) as ohp, \
             tc.tile_pool(name="epi", bufs=1) as epi, \
             tc.tile_pool(name="psum", bufs=1, space="PSUM") as psum:

            cst_s = consts.tile([128, CC], F16)
            # labs+iota (gates the first onehot build) land first; the
            # identities aren't needed until the compensator matmuls.
            nc.scalar.dma_start(cst_s[:, 0:nch + 128], cst_d[:, 0:nch + 128])
            nc.scalar.dma_start(cst_s[:, nch + 128:CC], cst_d[:, nch + 128:CC])
            labs_s = cst_s[:, 0:nch]
            iota_s = cst_s[:, nch:nch + 128]
            i128_s = cst_s[:, nch + 128:nch + 256]
            i64_s = cst_s[0:64, nch + 256:nch + 320]
            i72_s = cst_s[0:72, nch + 320:nch + 392]
            ident = {64: i64_s, 72: i72_s}
            ldc = consts.tile([128, 2, ROWS], F16)   # compensator chunks
            nc.scalar.dma_start(ldc[:], xc_d[:])

            acc = [psum.tile([BLK_WP[k], ROWS], F32, tag=f"acc{k}", bufs=1,
                             name=f"acc_{k}") for k in range(NBLK)]

            # PSUM tr tiles: [t-block, sample*roi] transposed raw-sum
            # rows; both samples share one bank per t-block.
            trA_ps = psum.tile([128, SPB * R], F16, tag="trA")
            trB_ps = psum.tile([72, SPB * R], F16, tag="trB")

            def finish_block(k, ms_eng):
                """Raw-sum epilogue for one ROI block: cast the PSUM sums
                to fp16 (Gram/transpose operand) and ship per-sample row
                sums (host applies the rank-1 centering correction)."""
                W = BLK_W[k]
                acc_ap = acc[k][0:W, :]
                S16 = epi.tile([W, ROWS], F16, tag=f"S16_{k}",
                               name=f"S16_{k}")
                ms = epi.tile([W, SPB], F32, tag=f"ms_{k}", name=f"ms_{k}")
                nc.vector.tensor_copy(S16[:], acc_ap)
                nc.vector.tensor_reduce(ms[:], _split_st(acc_ap),
                                        axis=mybir.AxisListType.X,
                                        op=mybir.AluOpType.add)
                ms_eng.dma_start(ms_d[k][:], ms[:])
                return S16

            def emit_transposes(k, S16):
                W = BLK_W[k]
                for s in range(SPB):
                    c0 = s * R + BW * k
                    nc.tensor.transpose(trA_ps[:, c0:c0 + W],
                                        S16[:, s * T:s * T + 128], ident[W])
                    nc.tensor.transpose(trB_ps[:, c0:c0 + W],
                                        S16[:, s * T + 128:s * T + 200],
                                        ident[W])

            S16s = {}
            pending_tr = {}                          # blk -> emit-at tile
            with nc.named_scope("main"):
                ch0 = 0
                for ti, ct in enumerate(tsizes):
                    ld = loads.tile([128, ct, ROWS], F8, tag=f"ld{ct}",
                                    bufs=(16 if ct == 8 else 4),
                                    name=f"ld_{ti}")
                    eng = nc.sync if ring_of[ti] == 0 else nc.scalar
                    eng.dma_start(ld[:], x_d[:, ch0:ch0 + ct, :])

                    # tile segments by block (tiles may straddle blocks)
                    segs = []                        # (k, jl0, cnt)
                    j = 0
                    while j < ct:
                        k = blk_of_chunk[ch0 + j]
                        cnt = 1
                        while j + cnt < ct and blk_of_chunk[ch0 + j + cnt] == k:
                            cnt += 1
                        segs.append((k, j, cnt))
                        j += cnt

                    # batched per-segment onehot builds (DVE, fp16
                    # compare -> fp8 out)
                    ohs = {}
                    for k, jl0, cnt in segs:
                        Wp = BLK_WP[k]
                        oh = ohp.tile([128, cnt, Wp], F8,
                                      tag=f"oh{k}_{cnt}", bufs=4,
                                      name=f"oh_{ti}_{k}")
                        nc.vector.tensor_tensor(
                            oh[:], _bc3(iota_s[:, 0:Wp], 1, cnt),
                            _bc3(labs_s[:, ch0 + jl0:ch0 + jl0 + cnt], 2, Wp),
                            op=mybir.AluOpType.is_equal)
                        ohs[(k, jl0)] = oh

                    # chunk-pair DoubleRow matmuls (2 chunks = 256
                    # contraction rows per instruction); odd segment
                    # leftovers fall back to single normal-mode matmuls.
                    for k, jl0, cnt in segs:
                        oh = ohs[(k, jl0)]
                        j = 0
                        while j < cnt:
                            start = (ch0 + jl0 + j == chunk0[k])
                            jl = jl0 + j
                            if j + 1 < cnt:
                                nc.tensor.matmul(
                                    acc[k][:], oh[:, j:j + 2, :],
                                    ld[:, jl:jl + 2, :],
                                    start=start, stop=False,
                                    perf_mode=mybir.MatmulPerfMode.DoubleRow)
                                j += 2
                            else:
                                nc.tensor.matmul(acc[k][:], oh[:, j, :],
                                                 ld[:, jl, :],
                                                 start=start, stop=False)
                                j += 1
                        if ch0 + jl0 + cnt - 1 == last_chunk[k]:
                            # identity-weight fp16 compensator matmul
                            # closes block k.
                            ci, c0, c1 = comp_map[k]
                            nc.tensor.matmul(acc[k][0:BLK_W[k], :],
                                             i128_s[:, c0:c1], ldc[:, ci, :],
                                             start=False, stop=True)
                            if k != 0:
                                S16s[k] = finish_block(k, nc.sync)
                                pending_tr[k] = ti + 3
                    ch0 += ct

                    for k, at in list(pending_tr.items()):
                        if ti == at:
                            # transposes a few tiles after the block's
                            # cast so PE's FIFO never blocks on it.
                            emit_transposes(k, S16s[k])
                            del pending_tr[k]

            with nc.named_scope("epilogue"):
                for k, S16 in S16s.items():
                    if k in pending_tr:
                        emit_transposes(k, S16)      # stream ended early
                        del pending_tr[k]
                # block-0 finish + transposes; the row-sum reduce follows
                # the cast directly in the DVE queue.
                S16_0 = finish_block(0, nc.sync)
                emit_transposes(0, S16_0)
                tr_sb = {}
                for s in range(SPB):
                    trA_sb = epi.tile([128, R], F16, name=f"trAs_{s}",
                                      tag="trAs", bufs=2)
                    trB_sb = epi.tile([72, R], F16, name=f"trBs_{s}",
                                      tag="trBs", bufs=2)
                    nc.vector.tensor_copy(trA_sb[:],
                                          trA_ps[:, s * R:(s + 1) * R])
                    nc.vector.tensor_copy(trB_sb[:],
                                          trB_ps[:, s * R:(s + 1) * R])
                    tr_sb[s] = (trA_sb, trB_sb)

                # Gram: conn = S_t.T @ S_t (contraction over t, fp16);
                # four independent PSUM banks so no Gram matmul waits on
                # a cast reading another sample's bank. cB DMAs ship all
                # 128 partitions (rows 72:128 are junk the host ignores)
                # - full-height transfers issue ~2x faster than 72-row.
                cA = [psum.tile([128, R], F32, tag=f"cA{s}",
                                name=f"cA_{s}") for s in range(SPB)]
                cB_ps = psum.tile([72, SPB * R], F32, tag="cB")
                connsb = epi.tile([128, 4 * R], F16, tag="connsb")
                for s in range(SPB):
                    trA_sb, trB_sb = tr_sb[s]
                    nc.tensor.matmul(cA[s][:], trA_sb[:, 0:128], trA_sb[:],
                                     start=True, stop=False)
                    nc.tensor.matmul(cA[s][:], trB_sb[:, 0:128], trB_sb[:],
                                     start=False, stop=True)
                    nc.vector.tensor_copy(connsb[:, s * R:(s + 1) * R],
                                          cA[s][:])
                    nc.sync.dma_start(out_d[:, s * R:(s + 1) * R],
                                      connsb[:, s * R:(s + 1) * R])
                for s in range(SPB):
                    trA_sb, trB_sb = tr_sb[s]
                    nc.tensor.matmul(cB_ps[:, s * R:(s + 1) * R],
                                     trA_sb[:, 128:200], trA_sb[:],
                                     start=True, stop=False)
                    nc.tensor.matmul(cB_ps[:, s * R:(s + 1) * R],
                                     trB_sb[:, 128:200], trB_sb[:],
                                     start=False, stop=True)
                nc.vector.tensor_copy(connsb[0:72, 2 * R:4 * R], cB_ps[:])
                nc.scalar.dma_start(out_d[:, 2 * R:4 * R],
                                    connsb[:, 2 * R:4 * R])

    nc.compile()
    return nc


def _get_program(ns):
    key = tuple(ns)
    if key not in _cached:
        _cached[key] = _build_program(list(ns))
    return _cached[key]


def marshal_inputs(x, parc, mask):
    """Host-side prep: ROI-sorted fp8 pixels + fp16 compensators whose
    values carry the summed fp8 quantization errors of their ROI, so
    per-ROI sums on device are fp16-exact."""
    import ml_dtypes

    parc_eff = np.where(np.asarray(mask), np.asarray(parc), 0).reshape(V)
    lab = parc_eff.astype(np.int64) - 1          # -1 = dropped
    counts = np.bincount(parc_eff.astype(np.int64), minlength=R + 1)[1:]
    assert counts.min() >= 2, "compensator scheme needs >=2 pixels per ROI"

    order = np.argsort(lab, kind="stable")
    nbg = int((lab < 0).sum())
    sorted_idx = order[nbg:]                     # kept pixels, ROI-ascending
    K = sorted_idx.size
    labk = lab[sorted_idx]
    comp_pos = np.searchsorted(labk, np.arange(R))  # first pixel per ROI
    rest_mask = np.ones(K, bool)
    rest_mask[comp_pos] = False
    rest_sorted = sorted_idx[rest_mask]          # ROI-sorted non-compensators
    lab_rest = labk[rest_mask]
    rb = np.searchsorted(lab_rest, np.arange(R))  # rest ROI start offsets
    comp_idx = sorted_idx[comp_pos]              # (R,) pixel index per ROI

    # per-block chunk counts over rest pixels (block 2 = rois 128..199)
    blk_of = np.minimum(lab_rest // BW, NBLK - 1)
    cs = [int((blk_of == k).sum()) for k in range(NBLK)]
    ns = [(c + 127) // 128 for c in cs]
    nch = sum(ns)

    # pack labels + gather indices in stream order
    labs_parts, rest_parts = [], []
    for k in SORDER:
        sel = blk_of == k
        labs_parts.append(lab_rest[sel] - BW * k)
        labs_parts.append(np.full(ns[k] * 128 - cs[k], -1, dtype=np.int64))
        rest_parts.append(rest_sorted[sel])
    labs = np.concatenate(labs_parts).astype(np.float16)
    labs = labs.reshape(nch, 128).T.copy()       # (128, nch)

    iota = np.broadcast_to(np.arange(128, dtype=np.float16),
                           (128, 128)).copy()    # iota[p, c] = c
    i128 = np.eye(128, dtype=np.float16)
    i64 = np.zeros((128, 64), dtype=np.float16)
    i64[:64] = np.eye(64, dtype=np.float16)
    i72 = np.zeros((128, 72), dtype=np.float16)
    i72[:72] = np.eye(72, dtype=np.float16)
    consts = np.concatenate([labs, iota, i128, i64, i72], axis=1)

    # quantize + compensate per sample (bounds transient memory)
    x32 = np.asarray(x, dtype=np.float32).reshape(N, T, V)
    xq8 = np.zeros((N, T, nch * 128), dtype=ml_dtypes.float8_e4m3fn)
    ycomp = np.empty((N, T, R), np.float16)
    # column ranges of each stream-ordered block in the packed array
    col0 = {}
    c = 0
    for k in SORDER:
        col0[k] = c
        c += ns[k] * 128
    for n in range(N):
        xr = x32[n][:, rest_sorted]              # (T, K-R) ROI-sorted
        q = xr.astype(ml_dtypes.float8_e4m3fn)
        e = xr.astype(np.float64) - q.astype(np.float64)
        esum = np.add.reduceat(e, rb, axis=1)    # (T, R) per-ROI error sums
        ycomp[n] = (x32[n][:, comp_idx].astype(np.float64) + esum
                    ).astype(np.float16)
        for k in SORDER:
            qk = q[:, blk_of == k]
            xq8[n, :, col0[k]:col0[k] + cs[k]] = qk

    # (N, T, nch*128) fp8 -> packed (core, 128, nch, SPB*T)
    xg = xq8.reshape(NCORES, SPB, T, nch, 128)
    xs = np.ascontiguousarray(xg.transpose(0, 4, 3, 1, 2))  # (8,128,nch,2,T)
    xs = xs.reshape(NCORES, 128, nch, ROWS)

    # compensator chunks (N, T, 2, 128):
    # chunk 0 = [blk2 comps (64) | blk3 comps (8) | 0], chunk 1 =
    # [blk1 comps (64) | blk0 comps (64)]
    xc = np.zeros((N, T, 2, 128), np.float16)
    xc[:, :, 0, 0:64] = ycomp[:, :, 128:192]
    xc[:, :, 0, 64:72] = ycomp[:, :, 192:200]
    xc[:, :, 1, 0:64] = ycomp[:, :, 64:128]
    xc[:, :, 1, 64:128] = ycomp[:, :, 0:64]
    xcg = xc.reshape(NCORES, SPB, T, 2, 128)
    xcs = np.ascontiguousarray(xcg.transpose(0, 4, 3, 1, 2))  # (8,128,2,2,T)
    xcs = xcs.reshape(NCORES, 128, 2, ROWS)

    in_maps = []
    for c in range(NCORES):
        in_maps.append({"x8": xs[c], "xc": xcs[c], "consts": consts})
    return in_maps, ns, counts


def kernel(x, parc, mask):
    in_maps, ns, counts = marshal_inputs(x, parc, mask)
    nc = _get_program(ns)
    res = run_bass_kernel_spmd(nc, in_maps, core_ids=list(range(NCORES)))
    # device emits the raw-sum Gram (fp16) + per-sample row sums; the
    # centering is a host-side rank-1 correction (C C^T = S S^T - m m^T/T
    # with m = row sums), and normalization a rank-1 scaling.
    G = np.empty((NCORES, SPB, R, R), np.float64)
    for c, r in enumerate(res.results):
        c2 = r["conn2"].astype(np.float64)       # (128, 800)
        for s in range(SPB):
            G[c, s, 0:128] = c2[:, s * R:(s + 1) * R]
            G[c, s, 128:R] = c2[0:72, (2 + s) * R:(3 + s) * R]
    G = G.reshape(N, R, R)
    ms = np.concatenate(
        [np.concatenate([r[f"ms{k}"] for k in range(NBLK)], axis=0)[None]
         for r in res.results], axis=0)           # (8, 200, SPB)
    ms = ms.transpose(0, 2, 1).reshape(N, R).astype(np.float64)  # (16, 200)
    G -= ms[:, :, None] * ms[:, None, :] / T
    d = np.einsum('nrr->nr', G)                   # ||C_r||^2
    rinv = 1.0 / (np.sqrt(d) + counts[None, :] * EPS)
    conn = G * rinv[:, :, None] * rinv[:, None, :]
    row, col = np.triu_indices(R, k=1)
    return np.ascontiguousarray(conn[:, row, col]).astype(np.float32)
